# revision 1
# baseline (speedup 1.0000x reference)
"""2-layer GraphSAGE (mean aggregation) on 8 Trainium2 NeuronCores.

CAGNET-style 1.5D sharding: adjacency rows (dst nodes) and node features are
sharded across the 8 cores (12544 padded nodes each); the small weight
matrices are replicated; the layer-1 activations are exchanged with chunked
AllGather collectives between the layers.

Device algorithm, per core and per 128-dst-node block:
  - src-feature rows are fetched with the GPSIMD `dma_gather` custom DMA
    (int16 indices; features live in DRAM permuted into "allgather space" so
    one index array addresses both x and the layer-1 AllGather output, split
    into 4 row-ranges of 25088 so indices stay within int16),
  - scatter-add into PSUM via one-hot matmuls (one-hot built on DVE by
    comparing dst-local ids against an iota row, exact in bf16),
  - 1/deg row scaling fused into the PSUM->SBUF copy as a per-partition
    ACT scale,
  - dense W_neigh/W_self matmuls run in feature-major orientation so the
    biases fuse into per-partition ACT bias and relu,
  - outputs written feature-major; the host un-permutes slots at the end.

All inputs are padded/sorted/permuted on the host (numpy); one SPMD program
serves all 8 cores because per-core data is laid out slot-major with shared
per-slot chunk counts.
"""
import os
import sys
import time

sys.path.insert(0, "/opt/trn_rl_repo")
import numpy as np
import ml_dtypes
import concourse.bass as bass  # noqa: E402
import concourse.tile as tile  # noqa: E402
from concourse import bacc, mybir  # noqa: E402
from concourse.library_config import mlp  # noqa: E402
from concourse.masks import make_identity  # noqa: E402

P = 128
NCORES = 8
N = 100000
NPAD = 100352                  # 784 blocks of 128
AGK = 7                        # allgather chunks (98 slots / 14)
RANGE = 25088                  # dma_gather int16-safe row range (4*25088=NPAD)
BF16 = mybir.dt.bfloat16
F32 = mybir.dt.float32
I16 = mybir.dt.int16
bf16 = ml_dtypes.bfloat16
PAD_DLOC = 200.0               # padded edge slots compare to nothing


def _wrap_idx(flat):
    w = flat.reshape(-1, 16).T
    return np.tile(w, (8, 1)).astype(np.int16)


def _preprocess(x, edge_index, Ws, bs):
    nblocks = NPAD // P
    S = nblocks // NCORES
    spc = S // AGK
    ns = S * P
    nrange = NPAD // RANGE

    src = edge_index[0].astype(np.int64)
    dst = edge_index[1].astype(np.int64)
    deg = np.bincount(dst, minlength=NPAD).astype(np.float64)
    invdeg = (1.0 / np.maximum(deg, 1.0)).astype(np.float32)

    order = np.argsort(dst, kind="stable")
    src_s = src[order]
    dst_s = dst[order]
    bounds = np.searchsorted(dst_s, np.arange(0, NPAD + 1, P))
    counts = bounds[1:] - bounds[:-1]

    slots, slot_of = [], []
    for c in range(NCORES):
        gbs = np.arange(c * S, (c + 1) * S)
        o = np.argsort(-counts[gbs], kind="stable")
        slots.append(gbs[o])
        inv = np.empty(S, np.int64)
        inv[o] = np.arange(S)
        slot_of.append(inv)

    agpos = np.empty(NPAD, np.int64)
    nodes = np.arange(NPAD)
    r = nodes // ns
    l = nodes % ns
    for c in range(NCORES):
        m = r == c
        s_owner = slot_of[c][l[m] // P]
        agpos[m] = ((s_owner // spc) * (NCORES * spc * P)
                    + c * (spc * P) + (s_owner % spc) * P + (l[m] % P))

    per_cs = []
    cntmax = np.zeros((S, nrange), np.int64)
    for c in range(NCORES):
        rows = []
        for s in range(S):
            gb = slots[c][s]
            lo, hi = bounds[gb], bounds[gb + 1]
            asrc = agpos[src_s[lo:hi]]
            dloc = dst_s[lo:hi] - gb * P
            gsel = asrc // RANGE
            groups = []
            for g in range(nrange):
                m = gsel == g
                groups.append((asrc[m] - g * RANGE, dloc[m]))
                cntmax[s, g] = max(cntmax[s, g], m.sum())
            rows.append(groups)
        per_cs.append(rows)
    K = -(-cntmax // P)
    for s in range(S):
        if K[s].sum() == 0:
            K[s, 0] = 1
            cntmax[s, 0] = 1
    C = K.sum(axis=1).astype(int)
    T = int(C.sum())
    offs = np.concatenate([[0], np.cumsum(C)]).astype(int)
    calls = []
    for s in range(S):
        lst, o = [], int(offs[s])
        for g in range(nrange):
            if K[s, g] > 0:
                # valid-index count shared across cores; shorter cores pad with
                # idx 0 up to it, the rest of the last chunk is trailing -1
                # (skipped by the ucode, no descriptors). The first slots keep
                # full idx-0 padding so rotating gather tiles never expose
                # uninitialized SBUF to the (zero) one-hot columns.
                nvalid = (int(K[s, g]) * P if s < 16
                          else int(cntmax[s, g]))
                lst.append((g, o, int(K[s, g]), nvalid))
                o += int(K[s, g])
        calls.append(lst)

    x_pad = np.zeros((NPAD, P), np.float32)
    x_pad[:x.shape[0]] = x
    xa = np.zeros((NPAD, P), np.float32)
    xa[agpos] = x_pad
    xa = xa.astype(bf16)

    Wn1, Ws1, Wn2, Ws2 = Ws
    bn1, bs1, bn2, bs2 = bs
    wz = np.concatenate([Wn1.T, Ws1.T, Wn2.T, Ws2.T], axis=0).astype(bf16)
    bz = np.concatenate([bn1 + bs1, bn2 + bs2]).astype(np.float32)

    in_maps, node_orders = [], []
    for c in range(NCORES):
        iw = np.zeros((P, 8 * T), np.int16)
        dl = np.full((P, T), PAD_DLOC, bf16)
        for s in range(S):
            for (g, o, k, nvalid) in calls[s]:
                asrc, dloc = per_cs[c][s][g]
                cnt = len(asrc)
                fi = np.full(k * P, -1, np.int64)
                fd = np.full(k * P, int(PAD_DLOC), np.int64)
                fi[:cnt] = asrc
                fi[cnt:nvalid] = 0
                fd[:cnt] = dloc
                iw[:, 8 * o:8 * (o + k)] = _wrap_idx(fi.astype(np.int16))
                dl[:, o:o + k] = fd.reshape(k, P).T.astype(bf16)
        node_order = (slots[c][:, None] * P + np.arange(P)).ravel()
        node_orders.append(node_order)
        iv = invdeg[node_order].reshape(S, P).T.copy()
        xt = np.ascontiguousarray(x_pad[node_order].T).astype(bf16)
        in_maps.append({
            "xa": xa, "iw": iw, "dl": dl, "iv": iv, "xt": xt,
            "wz": wz, "bz": bz,
        })

    meta = dict(S=S, spc=spc, ns=ns, C=[int(v) for v in C], T=T,
                offs=[int(v) for v in offs], calls=calls,
                node_orders=node_orders)
    return in_maps, meta


def _build_nc(meta):
    S, spc, ns = meta["S"], meta["spc"], meta["ns"]
    C, T, offs, calls = meta["C"], meta["T"], meta["offs"], meta["calls"]
    CMAX = max(C)

    nc = bacc.Bacc("TRN2", target_bir_lowering=False, debug=False,
                   num_devices=NCORES)
    xa = nc.dram_tensor("xa", [NPAD, P], BF16, kind="ExternalInput").ap()
    iw = nc.dram_tensor("iw", [P, 8 * T], I16, kind="ExternalInput").ap()
    dl = nc.dram_tensor("dl", [P, T], BF16, kind="ExternalInput").ap()
    iv = nc.dram_tensor("iv", [P, S], F32, kind="ExternalInput").ap()
    xt = nc.dram_tensor("xt", [P, ns], BF16, kind="ExternalInput").ap()
    wz = nc.dram_tensor("wz", [4 * P, P], BF16, kind="ExternalInput").ap()
    bz = nc.dram_tensor("bz", [2 * P], F32, kind="ExternalInput").ap()
    y = nc.dram_tensor("y", [P, ns], F32, kind="ExternalOutput").ap()

    with tile.TileContext(nc) as tc:
        with (
            tc.tile_pool(name="const", bufs=1) as cp,
            tc.tile_pool(name="dram", bufs=1, space="DRAM") as dp,
            tc.tile_pool(name="gpool", bufs=5) as gp,
            tc.tile_pool(name="mpool", bufs=4) as mp,
            tc.tile_pool(name="spool", bufs=4) as sp,
            tc.tile_pool(name="pacc", bufs=3, space="PSUM") as pacc,
            tc.tile_pool(name="ptr", bufs=2, space="PSUM") as ptr,
            tc.tile_pool(name="pmm", bufs=3, space="PSUM") as pmm,
        ):
            nc.gpsimd.load_library(mlp)
            iota_t = cp.tile([P, CMAX * P], BF16)
            nc.gpsimd.iota(iota_t[:], pattern=[[0, CMAX], [1, P]], base=0,
                           channel_multiplier=0,
                           allow_small_or_imprecise_dtypes=True)
            ident = cp.tile([P, P], BF16)
            make_identity(nc, ident[:])
            wn1 = cp.tile([P, P], BF16)
            ws1 = cp.tile([P, P], BF16)
            wn2 = cp.tile([P, P], BF16)
            ws2 = cp.tile([P, P], BF16)
            for i, w_ in enumerate((wn1, ws1, wn2, ws2)):
                nc.sync.dma_start(w_[:], wz[i * P:(i + 1) * P, :])
            b1 = cp.tile([P, 1], F32)
            nc.sync.dma_start(b1[:], bz[0:P, None])
            b2 = cp.tile([P, 1], F32)
            nc.sync.dma_start(b2[:], bz[P:2 * P, None])
            iv_t = cp.tile([P, S], F32)
            nc.sync.dma_start(iv_t[:], iv[:])
            iw_t = cp.tile([P, 8 * T], I16)
            nc.sync.dma_start(iw_t[:], iw[:])
            dl_t = cp.tile([P, T], BF16)
            nc.sync.dma_start(dl_t[:], dl[:])
            xt_t = cp.tile([P, ns], BF16)
            nc.sync.dma_start(xt_t[:], xt[:])
            h1T_all = cp.tile([P, ns], BF16)

            h1b = dp.tile([ns, P], BF16)
            h1f = dp.tile([NPAD, P], BF16)

            def scatter_agg(s, src_dram, layer):
                cs = C[s]
                g = gp.tile([P, CMAX * P], BF16, tag=f"g{layer}",
                            name=f"g{layer}_{s}")
                for (grp, o, k, nvalid) in calls[s]:
                    lo = (o - offs[s]) * P
                    nc.gpsimd.dma_gather(
                        g[:, lo:lo + k * P].rearrange("p (c f) -> p c f", c=k),
                        src_dram[grp * RANGE:(grp + 1) * RANGE, :],
                        iw_t[:, 8 * o:8 * (o + k)],
                        k * P, nvalid, P, single_packet=False,
                    )
                m = mp.tile([P, CMAX * P], BF16, tag=f"m{layer}",
                            name=f"m{layer}_{s}")
                nc.vector.tensor_tensor(
                    out=m[:, :cs * P],
                    in0=dl_t[:, offs[s]:offs[s + 1]].unsqueeze(2)
                        .broadcast_to([P, cs, P]),
                    in1=iota_t[:, :cs * P].rearrange("p (c f) -> p c f", c=cs),
                    op=mybir.AluOpType.is_equal,
                )
                ps = pacc.tile([P, P], F32, tag="acc", name=f"acc{layer}_{s}")
                for cc in range(cs):
                    nc.tensor.matmul(
                        out=ps[:], lhsT=m[:, cc * P:(cc + 1) * P],
                        rhs=g[:, cc * P:(cc + 1) * P],
                        start=(cc == 0), stop=(cc == cs - 1),
                    )
                agg = sp.tile([P, P], BF16, tag=f"agg{layer}",
                              name=f"agg{layer}_{s}")
                nc.scalar.activation(agg[:], ps[:],
                                     mybir.ActivationFunctionType.Copy,
                                     scale=iv_t[:, s:s + 1])
                pt = ptr.tile([P, P], BF16, tag="tr", name=f"tr{layer}_{s}")
                nc.tensor.transpose(pt[:], agg[:], ident[:])
                aggT = sp.tile([P, P], BF16, tag=f"aggT{layer}",
                               name=f"aggT{layer}_{s}")
                nc.vector.tensor_copy(aggT[:], pt[:])
                return aggT

            for s in range(S):
                aggT = scatter_agg(s, xa, 1)
                ph = pmm.tile([P, P], F32, tag="mm", name=f"mm1_{s}")
                nc.tensor.matmul(out=ph[:], lhsT=wn1[:], rhs=aggT[:],
                                 start=True, stop=False)
                nc.tensor.matmul(out=ph[:], lhsT=ws1[:],
                                 rhs=xt_t[:, s * P:(s + 1) * P],
                                 start=False, stop=True)
                nc.scalar.activation(h1T_all[:, s * P:(s + 1) * P], ph[:],
                                     mybir.ActivationFunctionType.Relu,
                                     bias=b1[:], scale=1.0)
                pt2 = ptr.tile([P, P], BF16, tag="tr", name=f"trh_{s}")
                nc.tensor.transpose(pt2[:], h1T_all[:, s * P:(s + 1) * P],
                                    ident[:])
                h1n = sp.tile([P, P], BF16, tag="h1n", name=f"h1n_{s}")
                nc.vector.tensor_copy(h1n[:], pt2[:])
                nc.sync.dma_start(h1b[s * P:(s + 1) * P, :], h1n[:])
                if (s + 1) % spc == 0:
                    cch = s // spc
                    nc.gpsimd.collective_compute(
                        "AllGather", mybir.AluOpType.bypass,
                        replica_groups=[list(range(NCORES))],
                        ins=[h1b[cch * spc * P:(cch + 1) * spc * P, :]],
                        outs=[h1f[cch * NCORES * spc * P:
                                  (cch + 1) * NCORES * spc * P, :]],
                    )

            for s in range(S):
                aggT = scatter_agg(s, h1f, 2)
                po = pmm.tile([P, P], F32, tag="mm", name=f"mm2_{s}")
                nc.tensor.matmul(out=po[:], lhsT=wn2[:], rhs=aggT[:],
                                 start=True, stop=False)
                nc.tensor.matmul(out=po[:], lhsT=ws2[:],
                                 rhs=h1T_all[:, s * P:(s + 1) * P],
                                 start=False, stop=True)
                oT = sp.tile([P, P], F32, tag="oT", name=f"oT_{s}")
                nc.scalar.activation(oT[:], po[:],
                                     mybir.ActivationFunctionType.Identity,
                                     bias=b2[:], scale=1.0)
                nc.sync.dma_start(y[:, s * P:(s + 1) * P], oT[:])

    nc.compile()
    return nc


def _run_spmd(nc, in_maps, n_timed=0):
    """Execute on the 8 cores via PJRT; optionally re-execute for timing.

    Returns (per-core result dicts, steady-state seconds or None).
    """
    import jax
    from jax.sharding import Mesh, PartitionSpec, NamedSharding
    from jax.experimental.shard_map import shard_map
    from concourse.bass2jax import (
        install_neuronx_cc_hook, _bass_exec_p, partition_id_tensor,
    )

    install_neuronx_cc_hook()
    partition_name = (nc.partition_id_tensor.name
                      if nc.partition_id_tensor else None)
    in_names, out_names, out_avals, zero_outs = [], [], [], []
    for alloc in nc.m.functions[0].allocations:
        if not isinstance(alloc, mybir.MemoryLocationSet):
            continue
        name = alloc.memorylocations[0].name
        if alloc.kind == "ExternalInput":
            if name != partition_name:
                in_names.append(name)
        elif alloc.kind == "ExternalOutput":
            shape = tuple(alloc.tensor_shape)
            dtype = mybir.dt.np(alloc.dtype)
            out_names.append(name)
            out_avals.append(jax.core.ShapedArray(shape, dtype))
            zero_outs.append(np.zeros(shape, dtype))
    n_params = len(in_names)
    n_outs = len(out_avals)
    in_names.extend(out_names)
    if partition_name is not None:
        in_names.append(partition_name)
    donate = tuple(range(n_params, n_params + n_outs))

    def _body(*args):
        operands = list(args)
        if partition_name is not None:
            operands.append(partition_id_tensor())
        return tuple(_bass_exec_p.bind(
            *operands, out_avals=tuple(out_avals), in_names=tuple(in_names),
            out_names=tuple(out_names), lowering_input_output_aliases=(),
            sim_require_finite=True, sim_require_nnan=True, nc=nc,
        ))

    devices = jax.devices()[:NCORES]
    mesh = Mesh(np.asarray(devices), ("core",))
    spec = NamedSharding(mesh, PartitionSpec("core"))
    sharded = jax.jit(
        shard_map(_body, mesh=mesh,
                  in_specs=(PartitionSpec("core"),) * (n_params + n_outs),
                  out_specs=(PartitionSpec("core"),) * n_outs,
                  check_rep=False),
        donate_argnums=donate, keep_unused=True,
    )
    per_core = [[np.asarray(m[name]) for name in in_names[:n_params]]
                for m in in_maps]
    concat_in = [np.concatenate([per_core[c][i] for c in range(NCORES)],
                                axis=0) for i in range(n_params)]
    dev_in = [jax.device_put(a, spec) for a in concat_in]
    jax.block_until_ready(dev_in)

    def make_zeros():
        zs = [jax.device_put(
            np.zeros((NCORES * z.shape[0], *z.shape[1:]), z.dtype), spec)
            for z in zero_outs]
        jax.block_until_ready(zs)
        return zs

    out_arrs = sharded(*dev_in, *make_zeros())
    jax.block_until_ready(out_arrs)

    t_exec = None
    if n_timed > 0:
        times = []
        for _ in range(n_timed):
            zs = make_zeros()
            t0 = time.time()
            out_arrs = sharded(*dev_in, *zs)
            jax.block_until_ready(out_arrs)
            times.append(time.time() - t0)
        t_exec = min(times)
    results = [
        {name: np.asarray(out_arrs[i]).reshape(NCORES, *out_avals[i].shape)[c]
         for i, name in enumerate(out_names)}
        for c in range(NCORES)
    ]
    return results, t_exec


def _make_runner(nc, in_maps):
    """Compile + pre-place inputs; return closure that times one execution."""
    import jax
    from jax.sharding import Mesh, PartitionSpec, NamedSharding
    from jax.experimental.shard_map import shard_map
    from concourse.bass2jax import (
        install_neuronx_cc_hook, _bass_exec_p, partition_id_tensor,
    )

    install_neuronx_cc_hook()
    partition_name = (nc.partition_id_tensor.name
                      if nc.partition_id_tensor else None)
    in_names, out_names, out_avals, zero_outs = [], [], [], []
    for alloc in nc.m.functions[0].allocations:
        if not isinstance(alloc, mybir.MemoryLocationSet):
            continue
        name = alloc.memorylocations[0].name
        if alloc.kind == "ExternalInput":
            if name != partition_name:
                in_names.append(name)
        elif alloc.kind == "ExternalOutput":
            out_names.append(name)
            out_avals.append(jax.core.ShapedArray(
                tuple(alloc.tensor_shape), mybir.dt.np(alloc.dtype)))
            zero_outs.append(np.zeros(tuple(alloc.tensor_shape),
                                      mybir.dt.np(alloc.dtype)))
    n_params = len(in_names)
    n_outs = len(out_avals)
    in_names.extend(out_names)
    if partition_name is not None:
        in_names.append(partition_name)
    donate = tuple(range(n_params, n_params + n_outs))

    def _body(*args):
        operands = list(args)
        if partition_name is not None:
            operands.append(partition_id_tensor())
        return tuple(_bass_exec_p.bind(
            *operands, out_avals=tuple(out_avals), in_names=tuple(in_names),
            out_names=tuple(out_names), lowering_input_output_aliases=(),
            sim_require_finite=True, sim_require_nnan=True, nc=nc,
        ))

    devices = jax.devices()[:NCORES]
    mesh = Mesh(np.asarray(devices), ("core",))
    spec = NamedSharding(mesh, PartitionSpec("core"))
    sharded = jax.jit(
        shard_map(_body, mesh=mesh,
                  in_specs=(PartitionSpec("core"),) * (n_params + n_outs),
                  out_specs=(PartitionSpec("core"),) * n_outs,
                  check_rep=False),
        donate_argnums=donate, keep_unused=True,
    )
    per_core = [[np.asarray(m[name]) for name in in_names[:n_params]]
                for m in in_maps]
    concat_in = [np.concatenate([per_core[c][i] for c in range(NCORES)],
                                axis=0) for i in range(n_params)]
    dev_in = [jax.device_put(a, spec) for a in concat_in]
    jax.block_until_ready(dev_in)

    def run_once():
        zs = [jax.device_put(
            np.zeros((NCORES * z.shape[0], *z.shape[1:]), z.dtype), spec)
            for z in zero_outs]
        jax.block_until_ready(zs)
        t0 = time.time()
        out = sharded(*dev_in, *zs)
        jax.block_until_ready(out)
        return time.time() - t0

    return run_once


def _null_baseline(n_timed):
    """Steady-state wall time of a trivial SPMD kernel — the fixed per-call
    dispatch overhead of this environment, used to estimate device time."""
    nc = bacc.Bacc("TRN2", target_bir_lowering=False, debug=False,
                   num_devices=NCORES)
    a = nc.dram_tensor("a0", [P, P], F32, kind="ExternalInput").ap()
    o = nc.dram_tensor("o0", [P, P], F32, kind="ExternalOutput").ap()
    with tile.TileContext(nc) as tc:
        with tc.tile_pool(name="sb", bufs=1) as sb:
            t = sb.tile([P, P], F32)
            nc.sync.dma_start(t[:], a[:])
            nc.sync.dma_start(o[:], t[:])
    nc.compile()
    _, t_null = _run_spmd(nc, [{"a0": np.zeros((P, P), np.float32)}] * NCORES,
                          n_timed=n_timed)
    return t_null


last_timing = {}


def kernel(**inputs):
    n_timed = int(os.environ.get("GNN_BENCH", "0"))
    x = np.asarray(inputs["x"], dtype=np.float32)
    edge_index = np.asarray(inputs["edge_index"])
    Ws = tuple(np.asarray(inputs[k], dtype=np.float32)
               for k in ("W_neigh1", "W_self1", "W_neigh2", "W_self2"))
    bs = tuple(np.asarray(inputs[k], dtype=np.float32)
               for k in ("b_neigh1", "b_self1", "b_neigh2", "b_self2"))

    in_maps, meta = _preprocess(x, edge_index, Ws, bs)
    nc = _build_nc(meta)
    results, t_exec = _run_spmd(nc, in_maps, n_timed=n_timed)

    if n_timed > 0:
        # interleave real/null samples so session-level dispatch drift cancels
        import jax
        nc0 = bacc.Bacc("TRN2", target_bir_lowering=False, debug=False,
                        num_devices=NCORES)
        a0 = nc0.dram_tensor("a0", [P, P], F32, kind="ExternalInput").ap()
        o0 = nc0.dram_tensor("o0", [P, P], F32, kind="ExternalOutput").ap()
        with tile.TileContext(nc0) as tc0:
            with tc0.tile_pool(name="sb", bufs=1) as sb0:
                t0_ = sb0.tile([P, P], F32)
                nc0.sync.dma_start(t0_[:], a0[:])
                nc0.sync.dma_start(o0[:], t0_[:])
        nc0.compile()
        null_in = [{"a0": np.zeros((P, P), np.float32)}] * NCORES
        reals, nulls = [], []
        real_runner = _make_runner(nc, in_maps)
        null_runner = _make_runner(nc0, null_in)
        real_runner(); null_runner()          # warm both
        for _ in range(n_timed):
            reals.append(real_runner())
            nulls.append(null_runner())
        t_exec = float(np.min(reals))
        t_null = float(np.min(nulls))
        last_timing["steady_s"] = t_exec
        last_timing["null_s"] = t_null
        last_timing["reals_ms"] = [round(v * 1e3, 2) for v in reals]
        last_timing["nulls_ms"] = [round(v * 1e3, 2) for v in nulls]
        last_timing["exec_ns"] = max(t_exec - t_null, 0.0) * 1e9

    y_full = np.zeros((NPAD, P), np.float32)
    for c in range(NCORES):
        y_full[meta["node_orders"][c], :] = results[c]["y"].T
    return y_full[:x.shape[0]]



# revision 2
# speedup vs baseline: 1.4910x; 1.4910x over previous
"""2-layer GraphSAGE (mean aggregation) on 8 Trainium2 NeuronCores — v2.

CAGNET-style 1.5D sharding as v1, restructured for the real-HW cost profile:
  - gather calls merged: one dma_gather per (batch of B dst blocks, source
    chunk group) instead of per (block, range) — ~8x fewer GPSIMD calls,
  - unified source grouping for both layers: 7 groups of 14336 rows; for
    layer 1 these are slices of the permuted x, for layer 2 they are exactly
    the 7 AllGather chunk tensors,
  - AllGather outputs are Shared-address-space DRAM tensors (single writer
    each), the fast HBM-HBM collective path,
  - gathers round-robin across SWDGE queues (configurable),
  - all padding gathers use index 0 (always-initialized rows; no -1/nvalid
    tricks), one-hot columns for padding slots are zero via dloc=PAD.

Per 128-dst-node block: scatter-add of gathered src rows into PSUM via
one-hot matmuls (one-hot built on DVE from dloc vs iota), 1/deg fused into
the PSUM->SBUF copy as per-partition ACT scale, dense W matmuls
feature-major with fused bias+relu, outputs written feature-major and
un-permuted on the host.
"""
import os
import sys
import time

sys.path.insert(0, "/opt/trn_rl_repo")
import numpy as np
import ml_dtypes
import concourse.bass as bass  # noqa: E402
import concourse.tile as tile  # noqa: E402
from concourse import bacc, mybir  # noqa: E402
from concourse.library_config import mlp  # noqa: E402
from concourse.masks import make_identity  # noqa: E402

P = 128
NCORES = 8
N = 100000
NPAD = 100352                  # 784 blocks of 128
S = NPAD // P // NCORES        # 98 blocks per core
AGK = 7                        # allgather chunks == gather source groups
SPC = S // AGK                 # 14 blocks per chunk
GROWS = NCORES * SPC * P       # 14336 rows per chunk tensor / source group
B = 7                          # dst blocks per merged gather call
NBATCH = S // B                # 14 batches
NQUEUES = 4                    # SWDGE queues for gather round-robin
BF16 = mybir.dt.bfloat16
F32 = mybir.dt.float32
I16 = mybir.dt.int16
bf16 = ml_dtypes.bfloat16
PAD_DLOC = 200.0               # padding edge slots compare equal to nothing


def _wrap_idx(flat):
    w = flat.reshape(-1, 16).T
    return np.tile(w, (8, 1)).astype(np.int16)


def _preprocess(x, edge_index, Ws, bs):
    src = edge_index[0].astype(np.int64)
    dst = edge_index[1].astype(np.int64)
    deg = np.bincount(dst, minlength=NPAD).astype(np.float64)
    invdeg = (1.0 / np.maximum(deg, 1.0)).astype(np.float32)

    order = np.argsort(dst, kind="stable")
    src_s = src[order]
    dst_s = dst[order]
    bounds = np.searchsorted(dst_s, np.arange(0, NPAD + 1, P))
    counts = bounds[1:] - bounds[:-1]

    # per-core slot ordering: blocks sorted by descending edge count so the
    # shared (max-over-core) chunk padding stays tight
    slots, slot_of = [], []
    for c in range(NCORES):
        gbs = np.arange(c * S, (c + 1) * S)
        o = np.argsort(-counts[gbs], kind="stable")
        slots.append(gbs[o])
        inv = np.empty(S, np.int64)
        inv[o] = np.arange(S)
        slot_of.append(inv)

    # node -> position in allgather space (chunk-major, then core, then slot)
    agpos = np.empty(NPAD, np.int64)
    nodes = np.arange(NPAD)
    r = nodes // (S * P)
    l = nodes % (S * P)
    for c in range(NCORES):
        m = r == c
        s_owner = slot_of[c][l[m] // P]
        agpos[m] = ((s_owner // SPC) * GROWS + c * (SPC * P)
                    + (s_owner % SPC) * P + (l[m] % P))

    # per (core, slot, group): local row indices + dst lane
    seg_idx = [[[None] * AGK for _ in range(S)] for _ in range(NCORES)]
    seg_dloc = [[[None] * AGK for _ in range(S)] for _ in range(NCORES)]
    cnt = np.zeros((NCORES, S, AGK), np.int64)
    for c in range(NCORES):
        for s in range(S):
            gb = slots[c][s]
            lo, hi = bounds[gb], bounds[gb + 1]
            asrc = agpos[src_s[lo:hi]]
            dloc = dst_s[lo:hi] - gb * P
            gsel = asrc // GROWS
            for g in range(AGK):
                m = gsel == g
                seg_idx[c][s][g] = asrc[m] - g * GROWS
                seg_dloc[c][s][g] = dloc[m]
                cnt[c, s, g] = m.sum()
    K = -(-cnt.max(axis=0) // P)          # [S, AGK] chunks, shared
    for s in range(S):
        if K[s].sum() == 0:
            K[s, 0] = 1

    # merged call list: per (batch, group) one dma_gather
    calls = []                            # (b, g, chunk_off, nchunks)
    Tg = 0
    for b in range(NBATCH):
        for g in range(AGK):
            k = int(K[b * B:(b + 1) * B, g].sum())
            if k:
                calls.append((b, g, Tg, k))
                Tg += k

    # block-major chunk space for dl / one-hot / matmul pairing
    cs_blk = K.sum(axis=1).astype(int)    # chunks per block
    dlo = np.concatenate([[0], np.cumsum(cs_blk)]).astype(int)
    Td = int(cs_blk.sum())
    assert Td == Tg

    # chunk map: for block s (in dl order) -> list of (call_idx, pos_in_call)
    call_at = {(b, g): i for i, (b, g, _, _) in enumerate(calls)}
    chunk_map = []                        # per s: list of (g, call_off+pos)
    for s in range(S):
        b = s // B
        lst = []
        for g in range(AGK):
            if K[s, g] == 0:
                continue
            ci = call_at[(b, g)]
            _, _, o, _ = calls[ci]
            pos = o + int(K[b * B:s, g].sum())
            for j in range(int(K[s, g])):
                lst.append((g, pos + j))
        chunk_map.append(lst)

    x_pad = np.zeros((NPAD, P), np.float32)
    x_pad[:x.shape[0]] = x
    xa = np.zeros((NPAD, P), np.float32)
    xa[agpos] = x_pad
    xa = xa.astype(bf16)

    Wn1, Ws1, Wn2, Ws2 = Ws
    bn1, bs1, bn2, bs2 = bs
    wz = np.concatenate([Wn1.T, Ws1.T, Wn2.T, Ws2.T], axis=0).astype(bf16)
    bz = np.concatenate([bn1 + bs1, bn2 + bs2]).astype(np.float32)

    in_maps, node_orders = [], []
    for c in range(NCORES):
        iw = np.zeros((P, 8 * Tg), np.int16)
        dl = np.full((P, Td), PAD_DLOC, bf16)
        for (b, g, o, k) in calls:
            flat = np.zeros(k * P, np.int64)
            pos = 0
            for s in range(b * B, (b + 1) * B):
                ks = int(K[s, g]) * P
                if ks == 0:
                    continue
                v = seg_idx[c][s][g]
                flat[pos:pos + len(v)] = v
                pos += ks
            iw[:, 8 * o:8 * (o + k)] = _wrap_idx(flat.astype(np.int16))
        for s in range(S):
            fd = np.full(cs_blk[s] * P, int(PAD_DLOC), np.int64)
            pos = 0
            for g in range(AGK):
                ks = int(K[s, g]) * P
                if ks == 0:
                    continue
                v = seg_dloc[c][s][g]
                fd[pos:pos + len(v)] = v
                pos += ks
            dl[:, dlo[s]:dlo[s] + cs_blk[s]] = (
                fd.reshape(cs_blk[s], P).T.astype(bf16))
        node_order = (slots[c][:, None] * P + np.arange(P)).ravel()
        node_orders.append(node_order)
        iv = invdeg[node_order].reshape(S, P).T.copy()
        # per-block diag(inv_deg): the PE "transpose" matmul agg.T @ diag
        # applies the 1/deg scale for free (ACT per-partition scale operands
        # are ~11us/call on real HW)
        dg = np.zeros((P, S * P), np.float32)
        lanes = np.arange(P)
        for s in range(S):
            dg[lanes, s * P + lanes] = iv[:, s]
        dg = dg.astype(bf16)
        xt = np.ascontiguousarray(x_pad[node_order].T).astype(bf16)
        in_maps.append({
            "xa": xa, "iw": iw, "dl": dl, "dg": dg, "xt": xt,
            "wz": wz, "bz": bz,
        })

    meta = dict(Tg=Tg, Td=Td, calls=calls, cs_blk=[int(v) for v in cs_blk],
                dlo=[int(v) for v in dlo], chunk_map=chunk_map,
                node_orders=node_orders)
    return in_maps, meta


def _build_nc(meta, nqueues=NQUEUES, shared_ag=True, local_copy=False):
    Tg, Td = meta["Tg"], meta["Td"]
    calls, cs_blk, dlo = meta["calls"], meta["cs_blk"], meta["dlo"]
    chunk_map = meta["chunk_map"]
    ns = S * P
    CSMAX = max(cs_blk)
    KBG = {}                              # widest call per group tag
    for (b, g, o, k) in calls:
        KBG[g] = max(KBG.get(g, 0), k)

    KBGM = max(KBG.values())

    nc = bacc.Bacc("TRN2", target_bir_lowering=False, debug=False,
                   num_devices=NCORES, num_swdge_queues=nqueues)
    xa = nc.dram_tensor("xa", [NPAD, P], BF16, kind="ExternalInput").ap()
    iw = nc.dram_tensor("iw", [P, 8 * Tg], I16, kind="ExternalInput").ap()
    dl = nc.dram_tensor("dl", [P, Td], BF16, kind="ExternalInput").ap()
    dg = nc.dram_tensor("dg", [P, S * P], BF16, kind="ExternalInput").ap()
    xt = nc.dram_tensor("xt", [P, ns], BF16, kind="ExternalInput").ap()
    wz = nc.dram_tensor("wz", [4 * P, P], BF16, kind="ExternalInput").ap()
    bz = nc.dram_tensor("bz", [2 * P], F32, kind="ExternalInput").ap()
    y = nc.dram_tensor("y", [P, ns], F32, kind="ExternalOutput").ap()

    with tile.TileContext(nc) as tc:
        with (
            tc.tile_pool(name="const", bufs=1) as cp,
            tc.tile_pool(name="dram", bufs=1, space="DRAM") as dp,
            tc.tile_pool(name="gpool", bufs=8) as gp,
            tc.tile_pool(name="mpool", bufs=4) as mp,
            tc.tile_pool(name="spool", bufs=4) as sp,
            tc.tile_pool(name="pacc", bufs=3, space="PSUM") as pacc,
            tc.tile_pool(name="ptr", bufs=2, space="PSUM") as ptr,
            tc.tile_pool(name="pmm", bufs=3, space="PSUM") as pmm,
        ):
            nc.gpsimd.load_library(mlp)
            iota2_t = cp.tile([P, CSMAX * P], BF16)
            nc.gpsimd.iota(iota2_t[:], pattern=[[1, P], [0, CSMAX]], base=0,
                           channel_multiplier=0,
                           allow_small_or_imprecise_dtypes=True)
            ident = cp.tile([P, P], BF16)
            make_identity(nc, ident[:])
            wn1 = cp.tile([P, P], BF16)
            ws1 = cp.tile([P, P], BF16)
            wn2 = cp.tile([P, P], BF16)
            ws2 = cp.tile([P, P], BF16)
            for i, w_ in enumerate((wn1, ws1, wn2, ws2)):
                nc.sync.dma_start(w_[:], wz[i * P:(i + 1) * P, :])
            b1 = cp.tile([P, 1], F32)
            nc.sync.dma_start(b1[:], bz[0:P, None])
            b2 = cp.tile([P, 1], F32)
            nc.sync.dma_start(b2[:], bz[P:2 * P, None])
            dg_t = cp.tile([P, S * P], BF16)
            nc.sync.dma_start(dg_t[:], dg[:])
            iw_t = cp.tile([P, 8 * Tg], I16)
            nc.sync.dma_start(iw_t[:], iw[:])
            dl_t = cp.tile([P, Td], BF16)
            nc.sync.dma_start(dl_t[:], dl[:])
            xt_t = cp.tile([P, ns], BF16)
            nc.sync.dma_start(xt_t[:], xt[:])
            h1T_all = cp.tile([P, ns], BF16)

            h1b = dp.tile([ns, P], BF16)
            if shared_ag:
                hcs = [dp.tile([GROWS, P], BF16, addr_space="Shared",
                               name=f"hc_{i}") for i in range(AGK)]
            else:
                hcs = [dp.tile([GROWS, P], BF16, name=f"hc_{i}")
                       for i in range(AGK)]

            qn = [0]

            def batch_gathers(b, layer, srcs):
                tiles = {}
                for (bb, g, o, k) in calls:
                    if bb != b:
                        continue
                    gt = gp.tile([P, KBGM * P], BF16, tag="g",
                                 name=f"g{layer}_{b}_{g}")
                    nc.gpsimd.dma_gather(
                        gt[:, :k * P].rearrange("p (c f) -> p c f", c=k),
                        srcs[g],
                        iw_t[:, 8 * o:8 * (o + k)],
                        k * P, k * P, P, single_packet=False,
                        queue_num=qn[0] % nqueues,
                    )
                    qn[0] += 1
                    tiles[g] = (gt, o)
                return tiles

            def block_agg(s, tiles, layer):
                cs = cs_blk[s]
                m = mp.tile([P, CSMAX * P], BF16, tag="m",
                            name=f"m{layer}_{s}")
                # m2 layout: m[p, f*cs + c] = (dl[p, c] == f); all
                # operands inner-packed -> DVE 2x mode
                nc.vector.tensor_tensor(
                    out=m[:, :cs * P].rearrange("p (f c) -> p f c", f=P),
                    in0=dl_t[:, dlo[s]:dlo[s] + cs].unsqueeze(1)
                        .broadcast_to([P, P, cs]),
                    in1=iota2_t[:].rearrange("p (f c) -> p f c",
                                             c=CSMAX)[:, :, :cs],
                    op=mybir.AluOpType.is_equal,
                )
                ps = pacc.tile([P, P], F32, tag="acc", name=f"acc{layer}_{s}")
                m3 = m[:, :cs * P].rearrange("p (f c) -> p f c", f=P)
                for j, (g, pos) in enumerate(chunk_map[s]):
                    gt, o = tiles[g]
                    lo = (pos - o) * P
                    nc.tensor.matmul(
                        out=ps[:], lhsT=m3[:, :, j],
                        rhs=gt[:, lo:lo + P],
                        start=(j == 0), stop=(j == cs - 1),
                    )
                agg = sp.tile([P, P], BF16, tag="agg", name=f"agg{layer}_{s}")
                nc.scalar.activation(agg[:], ps[:],
                                     mybir.ActivationFunctionType.Copy,
                                     scale=1.0)
                # transpose against diag(1/deg) applies the mean-normalization
                # on the PE for free: pt = agg.T @ diag. Must be a plain
                # matmul — the is_transpose PE mode ignores the rhs values.
                pt = ptr.tile([P, P], F32, tag="tr", name=f"tr{layer}_{s}")
                nc.tensor.matmul(out=pt[:], lhsT=agg[:],
                                 rhs=dg_t[:, s * P:(s + 1) * P],
                                 start=True, stop=True)
                aggT = sp.tile([P, P], BF16, tag="aggT",
                               name=f"aggT{layer}_{s}")
                nc.vector.tensor_copy(aggT[:], pt[:])
                return aggT

            # ---------------- layer 1 ----------------
            srcs1 = {g: xa[g * GROWS:(g + 1) * GROWS, :] for g in range(AGK)}
            for b in range(NBATCH):
                tiles = batch_gathers(b, 1, srcs1)
                for s in range(b * B, (b + 1) * B):
                    aggT = block_agg(s, tiles, 1)
                    ph = pmm.tile([P, P], F32, tag="mm", name=f"mm1_{s}")
                    nc.tensor.matmul(out=ph[:], lhsT=wn1[:], rhs=aggT[:],
                                     start=True, stop=False)
                    nc.tensor.matmul(out=ph[:], lhsT=ws1[:],
                                     rhs=xt_t[:, s * P:(s + 1) * P],
                                     start=False, stop=True)
                    nc.scalar.activation(h1T_all[:, s * P:(s + 1) * P],
                                         ph[:],
                                         mybir.ActivationFunctionType.Relu,
                                         bias=b1[:], scale=1.0)
                    pt2 = ptr.tile([P, P], BF16, tag="tr", name=f"trh_{s}")
                    nc.tensor.transpose(pt2[:],
                                        h1T_all[:, s * P:(s + 1) * P],
                                        ident[:])
                    h1n = sp.tile([P, P], BF16, tag="h1n", name=f"h1n_{s}")
                    nc.vector.tensor_copy(h1n[:], pt2[:])
                    nc.sync.dma_start(h1b[s * P:(s + 1) * P, :], h1n[:])
            # all AllGathers issued after the last L1 gather call: on the
            # in-order Pool queue a (blocking) collective can then never
            # stall an L1 gather; each AG still waits on its chunk's h1b
            # writes via semaphores and drains during the L1 compute tail
            for cch in range(AGK):
                nc.gpsimd.collective_compute(
                    "AllGather", mybir.AluOpType.bypass,
                    replica_groups=[list(range(NCORES))],
                    ins=[h1b[cch * SPC * P:(cch + 1) * SPC * P, :]],
                    outs=[hcs[cch][:]],
                )
            if local_copy:
                # gathers from Shared DRAM contend on one HBM partition;
                # stage each chunk into core-local DRAM first
                lcs = [dp.tile([GROWS, P], BF16, name=f"lc_{i}")
                       for i in range(AGK)]
                for g in range(AGK):
                    nc.sync.dma_start(lcs[g][:], hcs[g][:])
                l2src = lcs
            else:
                l2src = hcs

            # ---------------- layer 2 ----------------
            srcs2 = {g: l2src[g][:] for g in range(AGK)}
            for b in range(NBATCH):
                tiles = batch_gathers(b, 2, srcs2)
                for s in range(b * B, (b + 1) * B):
                    aggT = block_agg(s, tiles, 2)
                    po = pmm.tile([P, P], F32, tag="mm", name=f"mm2_{s}")
                    nc.tensor.matmul(out=po[:], lhsT=wn2[:], rhs=aggT[:],
                                     start=True, stop=False)
                    nc.tensor.matmul(out=po[:], lhsT=ws2[:],
                                     rhs=h1T_all[:, s * P:(s + 1) * P],
                                     start=False, stop=True)
                    oT = sp.tile([P, P], F32, tag="oT", name=f"oT_{s}")
                    nc.scalar.activation(oT[:], po[:],
                                         mybir.ActivationFunctionType.Identity,
                                         bias=b2[:], scale=1.0)
                    nc.sync.dma_start(y[:, s * P:(s + 1) * P], oT[:])

    nc.compile()
    return nc



def _run_spmd(nc, in_maps, n_timed=0):
    """Execute on the 8 cores via PJRT; optionally re-execute for timing.

    Returns (per-core result dicts, steady-state seconds or None).
    """
    import jax
    from jax.sharding import Mesh, PartitionSpec, NamedSharding
    from jax.experimental.shard_map import shard_map
    from concourse.bass2jax import (
        install_neuronx_cc_hook, _bass_exec_p, partition_id_tensor,
    )

    install_neuronx_cc_hook()
    partition_name = (nc.partition_id_tensor.name
                      if nc.partition_id_tensor else None)
    in_names, out_names, out_avals, zero_outs = [], [], [], []
    for alloc in nc.m.functions[0].allocations:
        if not isinstance(alloc, mybir.MemoryLocationSet):
            continue
        name = alloc.memorylocations[0].name
        if alloc.kind == "ExternalInput":
            if name != partition_name:
                in_names.append(name)
        elif alloc.kind == "ExternalOutput":
            shape = tuple(alloc.tensor_shape)
            dtype = mybir.dt.np(alloc.dtype)
            out_names.append(name)
            out_avals.append(jax.core.ShapedArray(shape, dtype))
            zero_outs.append(np.zeros(shape, dtype))
    n_params = len(in_names)
    n_outs = len(out_avals)
    in_names.extend(out_names)
    if partition_name is not None:
        in_names.append(partition_name)
    donate = tuple(range(n_params, n_params + n_outs))

    def _body(*args):
        operands = list(args)
        if partition_name is not None:
            operands.append(partition_id_tensor())
        return tuple(_bass_exec_p.bind(
            *operands, out_avals=tuple(out_avals), in_names=tuple(in_names),
            out_names=tuple(out_names), lowering_input_output_aliases=(),
            sim_require_finite=True, sim_require_nnan=True, nc=nc,
        ))

    devices = jax.devices()[:NCORES]
    mesh = Mesh(np.asarray(devices), ("core",))
    spec = NamedSharding(mesh, PartitionSpec("core"))
    sharded = jax.jit(
        shard_map(_body, mesh=mesh,
                  in_specs=(PartitionSpec("core"),) * (n_params + n_outs),
                  out_specs=(PartitionSpec("core"),) * n_outs,
                  check_rep=False),
        donate_argnums=donate, keep_unused=True,
    )
    per_core = [[np.asarray(m[name]) for name in in_names[:n_params]]
                for m in in_maps]
    concat_in = [np.concatenate([per_core[c][i] for c in range(NCORES)],
                                axis=0) for i in range(n_params)]
    dev_in = [jax.device_put(a, spec) for a in concat_in]
    jax.block_until_ready(dev_in)

    def make_zeros():
        zs = [jax.device_put(
            np.zeros((NCORES * z.shape[0], *z.shape[1:]), z.dtype), spec)
            for z in zero_outs]
        jax.block_until_ready(zs)
        return zs

    out_arrs = sharded(*dev_in, *make_zeros())
    jax.block_until_ready(out_arrs)

    t_exec = None
    if n_timed > 0:
        times = []
        for _ in range(n_timed):
            zs = make_zeros()
            t0 = time.time()
            out_arrs = sharded(*dev_in, *zs)
            jax.block_until_ready(out_arrs)
            times.append(time.time() - t0)
        t_exec = min(times)
    results = [
        {name: np.asarray(out_arrs[i]).reshape(NCORES, *out_avals[i].shape)[c]
         for i, name in enumerate(out_names)}
        for c in range(NCORES)
    ]
    return results, t_exec


def _make_runner(nc, in_maps):
    """Compile + pre-place inputs; return closure that times one execution."""
    import jax
    from jax.sharding import Mesh, PartitionSpec, NamedSharding
    from jax.experimental.shard_map import shard_map
    from concourse.bass2jax import (
        install_neuronx_cc_hook, _bass_exec_p, partition_id_tensor,
    )

    install_neuronx_cc_hook()
    partition_name = (nc.partition_id_tensor.name
                      if nc.partition_id_tensor else None)
    in_names, out_names, out_avals, zero_outs = [], [], [], []
    for alloc in nc.m.functions[0].allocations:
        if not isinstance(alloc, mybir.MemoryLocationSet):
            continue
        name = alloc.memorylocations[0].name
        if alloc.kind == "ExternalInput":
            if name != partition_name:
                in_names.append(name)
        elif alloc.kind == "ExternalOutput":
            out_names.append(name)
            out_avals.append(jax.core.ShapedArray(
                tuple(alloc.tensor_shape), mybir.dt.np(alloc.dtype)))
            zero_outs.append(np.zeros(tuple(alloc.tensor_shape),
                                      mybir.dt.np(alloc.dtype)))
    n_params = len(in_names)
    n_outs = len(out_avals)
    in_names.extend(out_names)
    if partition_name is not None:
        in_names.append(partition_name)
    donate = tuple(range(n_params, n_params + n_outs))

    def _body(*args):
        operands = list(args)
        if partition_name is not None:
            operands.append(partition_id_tensor())
        return tuple(_bass_exec_p.bind(
            *operands, out_avals=tuple(out_avals), in_names=tuple(in_names),
            out_names=tuple(out_names), lowering_input_output_aliases=(),
            sim_require_finite=True, sim_require_nnan=True, nc=nc,
        ))

    devices = jax.devices()[:NCORES]
    mesh = Mesh(np.asarray(devices), ("core",))
    spec = NamedSharding(mesh, PartitionSpec("core"))
    sharded = jax.jit(
        shard_map(_body, mesh=mesh,
                  in_specs=(PartitionSpec("core"),) * (n_params + n_outs),
                  out_specs=(PartitionSpec("core"),) * n_outs,
                  check_rep=False),
        donate_argnums=donate, keep_unused=True,
    )
    per_core = [[np.asarray(m[name]) for name in in_names[:n_params]]
                for m in in_maps]
    concat_in = [np.concatenate([per_core[c][i] for c in range(NCORES)],
                                axis=0) for i in range(n_params)]
    dev_in = [jax.device_put(a, spec) for a in concat_in]
    jax.block_until_ready(dev_in)

    def run_once():
        zs = [jax.device_put(
            np.zeros((NCORES * z.shape[0], *z.shape[1:]), z.dtype), spec)
            for z in zero_outs]
        jax.block_until_ready(zs)
        t0 = time.time()
        out = sharded(*dev_in, *zs)
        jax.block_until_ready(out)
        return time.time() - t0

    return run_once


def _null_baseline(n_timed):
    """Steady-state wall time of a trivial SPMD kernel — the fixed per-call
    dispatch overhead of this environment, used to estimate device time."""
    nc = bacc.Bacc("TRN2", target_bir_lowering=False, debug=False,
                   num_devices=NCORES)
    a = nc.dram_tensor("a0", [P, P], F32, kind="ExternalInput").ap()
    o = nc.dram_tensor("o0", [P, P], F32, kind="ExternalOutput").ap()
    with tile.TileContext(nc) as tc:
        with tc.tile_pool(name="sb", bufs=1) as sb:
            t = sb.tile([P, P], F32)
            nc.sync.dma_start(t[:], a[:])
            nc.sync.dma_start(o[:], t[:])
    nc.compile()
    _, t_null = _run_spmd(nc, [{"a0": np.zeros((P, P), np.float32)}] * NCORES,
                          n_timed=n_timed)
    return t_null


last_timing = {}



last_timing = {}


def kernel(**inputs):
    n_timed = int(os.environ.get("GNN_BENCH", "0"))
    x = np.asarray(inputs["x"], dtype=np.float32)
    edge_index = np.asarray(inputs["edge_index"])
    Ws = tuple(np.asarray(inputs[k], dtype=np.float32)
               for k in ("W_neigh1", "W_self1", "W_neigh2", "W_self2"))
    bs = tuple(np.asarray(inputs[k], dtype=np.float32)
               for k in ("b_neigh1", "b_self1", "b_neigh2", "b_self2"))

    in_maps, meta = _preprocess(x, edge_index, Ws, bs)
    nc = _build_nc(meta)
    results, t_exec = _run_spmd(nc, in_maps, n_timed=0)

    if n_timed > 0:
        # interleave real/null samples so session-level dispatch drift cancels
        nc0 = bacc.Bacc("TRN2", target_bir_lowering=False, debug=False,
                        num_devices=NCORES)
        a0 = nc0.dram_tensor("a0", [P, P], F32, kind="ExternalInput").ap()
        o0 = nc0.dram_tensor("o0", [P, P], F32, kind="ExternalOutput").ap()
        with tile.TileContext(nc0) as tc0:
            with tc0.tile_pool(name="sb", bufs=1) as sb0:
                t0_ = sb0.tile([P, P], F32)
                nc0.sync.dma_start(t0_[:], a0[:])
                nc0.sync.dma_start(o0[:], t0_[:])
        nc0.compile()
        null_in = [{"a0": np.zeros((P, P), np.float32)}] * NCORES
        reals, nulls = [], []
        real_runner = _make_runner(nc, in_maps)
        null_runner = _make_runner(nc0, null_in)
        real_runner(); null_runner()          # warm both
        for _ in range(n_timed):
            reals.append(real_runner())
            nulls.append(null_runner())
        t_exec = float(np.min(reals))
        t_null = float(np.min(nulls))
        last_timing["steady_s"] = t_exec
        last_timing["null_s"] = t_null
        last_timing["reals_ms"] = [round(v * 1e3, 2) for v in reals]
        last_timing["nulls_ms"] = [round(v * 1e3, 2) for v in nulls]
        last_timing["exec_ns"] = max(t_exec - t_null, 0.0) * 1e9

    y_full = np.zeros((NPAD, P), np.float32)
    for c in range(NCORES):
        y_full[meta["node_orders"][c], :] = results[c]["y"].T
    return y_full[:x.shape[0]]
